# revision 58
# baseline (speedup 1.0000x reference)
"""Octahedral SHT on 8 NeuronCores (Bass/Tile).

Strategy: shard the 192 latitude rings across 8 cores (24 rings each). Each
ring's ragged DFT (nlon in 20..400) is cut into K=128 chunks, zero-padded;
the ring assignment is engineered so every core gets exactly 51 chunks
-> one uniform SPMD program. The per-ring Legendre weights are replicated
per chunk, which folds the intra-ring chunk reduction into phase 2.
Each core computes a partial [l, m, bev] coefficient tensor over its own
rings; the host sums the 8 partials and assembles the complex output.

Precision: fp32 matmuls on the PE are 4x slower, so each fp32 operand is
split hi/lo into two fp16 tensors (x = hi + lo, |lo| <= 2^-11 |x|).
fp16*fp16 products are exact in the fp32 PSUM accumulator, so accumulating
MMs (hi*hi + hi*lo + lo*hi) reproduce the fp32 product to ~2^-22.

Phase 1 (per chunk c): G[c][m, (r,bev)] = E2[c].T @ x[c]  (PE, 6 MMs N=128,
  psum partition dim = m so the flatten yields m-major G' rows)
Flatten: G'[row c] <- [m, (hi|lo)] bounced through DRAM: SBUF->DRAM writes
  run at full HBM rate (the direct SBUF->SBUF gather is wall-limited by
  single-partition write bandwidth), then G' loads back in m-quarters whose
  SBUF writes spread across all 51 partitions, pipelined with phase-2 MMs.
  Row layout m*512 + h*256 + r*128 + bev.
Phase 2 (per m): out[l, (r,bev)] = 3 MMs (K=51):
  pw_hi @ G'hi (start) ; pw_hi @ G'lo ; pw_lo @ G'hi (stop)
"""
import numpy as np

NLAT, LMAX, MMAX = 192, 128, 128
B, V = 2, 64
BF = B * V            # 128 fused batch (b*64+v)
NCORES = 8
CHUNK = 128
NCH = 51              # chunks per core
RINGS_PER_CORE = 24
MAX_NLON = 400
NPTS = 40320
GB = [0, 4, 7, 10, 13, 17, 20, 23, 26, 30, 33, 36, 39, 43, 46, 49, NCH]
MG = 4                      # m's per psum tile
OG = 8                      # m's per out DMA group
PWG = 16                    # m's per pw/G' load group


def _octa_nlon():
    half = NLAT // 2
    north = np.array([4 * (i + 1) + 16 for i in range(half)], dtype=np.int64)
    return np.concatenate([north, north[::-1]])


def _ring_assignment():
    nlon = _octa_nlon()
    v = np.ceil(nlon / CHUNK).astype(int)
    cores = [[] for _ in range(NCORES)]
    for cls in (1, 2, 3, 4):
        ids = np.where(v == cls)[0]
        ids = ids[np.argsort(-nlon[ids], kind="stable")]
        fwd = True
        for start in range(0, len(ids), NCORES):
            blk = ids[start:start + NCORES]
            order = range(NCORES) if fwd else range(NCORES - 1, -1, -1)
            for c, rid in zip(order, blk):
                cores[c].append(int(rid))
            fwd = not fwd
    return cores, nlon


def _split16(a):
    hi = a.astype(np.float16)
    lo = (a - hi.astype(np.float32)).astype(np.float16)
    return hi, lo


def _build_core_inputs(core_rings, nlon, offs, x, E_re, E_im, PwT):
    """x: [BF, npts] f32.  Returns:
    xe  [128 j, 51 c, 768] f16  cols: [x_hi | x_lo | Ehi_re | Ehi_im | Elo_re | Elo_im]
    pw  [51, 128 m, 256] f16    cols: [pw_hi 0:128 | pw_lo 128:256]
    """
    xpad = np.zeros((NCH, CHUNK, BF), np.float32)
    E2 = np.zeros((NCH, CHUNK, 2 * MMAX), np.float32)
    Pw2 = np.zeros((MMAX, NCH, LMAX), np.float32)
    c = 0
    for r in core_rings:
        nl = int(nlon[r])
        o = int(offs[r])
        for j0 in range(0, nl, CHUNK):
            jlen = min(CHUNK, nl - j0)
            xpad[c, :jlen, :] = x[:, o + j0:o + j0 + jlen].T
            elen = min(CHUNK, MAX_NLON - j0)
            if elen > 0:
                E2[c, :elen, 0:MMAX] = E_re[r, j0:j0 + elen, :]
                E2[c, :elen, MMAX:] = E_im[r, j0:j0 + elen, :]
            Pw2[:, c, :] = PwT[:, r, :]
            c += 1
    assert c == NCH
    xh, xl = _split16(xpad)
    eh, el = _split16(E2)
    # eh/el cols: [re 0:128 | im 128:256]
    xe = np.concatenate([xh, xl, eh[:, :, 0:128], eh[:, :, 128:256],
                         el[:, :, 0:128], el[:, :, 128:256]], axis=2)
    xe = np.ascontiguousarray(xe.transpose(1, 0, 2))  # [128 j, 51 c, 768]

    ph, pl = _split16(Pw2)                           # [m, c, l] each
    pw = np.zeros((NCH, MMAX, 2 * LMAX), np.float16)
    pw[:, :, 0:128] = ph.transpose(1, 0, 2)          # pw_hi
    pw[:, :, 128:256] = pl.transpose(1, 0, 2)        # pw_lo
    return xe, pw


def _build_bass():
    import concourse.bass as bass
    import concourse.mybir as mybir
    from concourse import bacc, tile

    dt = mybir.dt
    nc = bacc.Bacc()

    xe_d = nc.dram_tensor("xe", [CHUNK, NCH, 768], dt.float16,
                          kind="ExternalInput")
    pw_d = nc.dram_tensor("pw", [NCH, MMAX, 2 * LMAX], dt.float16,
                          kind="ExternalInput")
    outp_d = nc.dram_tensor("outp", [LMAX, MMAX, 2 * BF], dt.float32,
                            kind="ExternalOutput")
    gdram = nc.dram_tensor("gdram", [NCH, MMAX * 512], dt.float16)

    with tile.TileContext(nc) as tc:
        with (
            tc.tile_pool(name="xs", bufs=4) as xs_pool,
            tc.tile_pool(name="gt", bufs=4) as gt_pool,
            tc.tile_pool(name="gs", bufs=4) as gs_pool,
            tc.tile_pool(name="pws", bufs=4) as pw_pool,
            tc.tile_pool(name="os", bufs=2) as os_pool,
            tc.tile_pool(name="ps1", bufs=2, space="PSUM") as ps1,
            tc.tile_pool(name="ps2", bufs=3, space="PSUM") as ps2,
        ):
            # ---- phase 1: 51 chunks x 6 accumulating MMs ----
            NG = len(GB) - 1
            xg = {}
            for g in range(NG):
                n = GB[g + 1] - GB[g]
                t = xs_pool.tile([CHUNK, n * 768], dt.float16, tag="xg")
                # split by partition halves across rotating queues
                e1 = (nc.sync, nc.scalar, nc.gpsimd)[g % 3]
                e2 = (nc.scalar, nc.gpsimd, nc.sync)[g % 3]
                e1.dma_start(out=t[0:64, :],
                             in_=xe_d[0:64, GB[g]:GB[g + 1], :])
                e2.dma_start(out=t[64:128, :],
                             in_=xe_d[64:128, GB[g]:GB[g + 1], :])
                xg[g] = t

            for c in range(NCH):
                g = next(i for i in range(NG) if GB[i] <= c < GB[i + 1])
                off = (c - GB[g]) * 768
                xe = xg[g]
                xh = xe[:, off + 0:off + 128]
                xl = xe[:, off + 128:off + 256]
                ehr = xe[:, off + 256:off + 384]
                ehi = xe[:, off + 384:off + 512]
                elr = xe[:, off + 512:off + 640]
                eli = xe[:, off + 640:off + 768]
                # psum [m, (re_bev | im_bev)]; 6 MMs, lhsT = E slices
                g_ps = ps1.tile([MMAX, 2 * BF], dt.float32, tag="g")
                re = g_ps[:, 0:128]
                im = g_ps[:, 128:256]
                nc.tensor.matmul(re, ehr, xh, start=True, stop=False)
                nc.tensor.matmul(re, ehr, xl, start=False, stop=False)
                nc.tensor.matmul(re, elr, xh, start=False, stop=True)
                nc.tensor.matmul(im, ehi, xh, start=True, stop=False)
                nc.tensor.matmul(im, ehi, xl, start=False, stop=False)
                nc.tensor.matmul(im, eli, xh, start=False, stop=True)
                # evacuate PSUM, splitting fp32 -> fp16 hi (ACT) + lo (DVE)
                g_hl = gt_pool.tile([MMAX, 512], dt.float16, tag="ghl")
                nc.scalar.copy(g_hl[:, 0:256], g_ps[:])
                nc.vector.tensor_sub(g_hl[:, 256:512], g_ps[:], g_hl[:, 0:256])
                # flatten to DRAM (full-rate HBM write, 1KB runs)
                nc.gpsimd.dma_start(out=gdram[c], in_=g_hl[:])

            # ---- phase 2: 128 m x 3 accumulating MMs ----
            for mg in range(0, MMAX, PWG):
                pwt = pw_pool.tile([NCH, PWG * 256], dt.float16, tag="pw")
                nc.sync.dma_start(out=pwt[:], in_=pw_d[:, mg:mg + PWG, :])
                gsb = gs_pool.tile([NCH, PWG * 512], dt.float16, tag="gq")
                # parallel loads (8KB runs each) split across queues; the
                # first group is on the critical path -> split 4 ways
                nsplit = 4 if mg == 0 else 2
                step = PWG * 512 // nsplit
                engs = (nc.sync, nc.gpsimd, nc.scalar, nc.sync)
                for si in range(nsplit):
                    engs[si].dma_start(
                        out=gsb[:, si * step:(si + 1) * step],
                        in_=gdram[:, mg * 512 + si * step:
                                  mg * 512 + (si + 1) * step])
                for m8 in range(mg, mg + PWG, OG):
                    o_sb = os_pool.tile([LMAX, OG * 256], dt.float32, tag="ot")
                    for m4 in range(m8, m8 + OG, MG):
                        o_ps = ps2.tile([LMAX, MG * 256], dt.float32, tag="o")
                        for m in range(m4, m4 + MG):
                            mo = (m - mg) * 256
                            po = (m - m4) * 256
                            ml = (m - mg) * 512
                            pa = pwt[:, mo + 0:mo + 128]
                            pb = pwt[:, mo + 128:mo + 256]
                            rhs_h = gsb[:, ml:ml + 256]
                            rhs_l = gsb[:, ml + 256:ml + 512]
                            nc.tensor.matmul(o_ps[:, po:po + 256], pa,
                                             rhs_h, start=True, stop=False)
                            nc.tensor.matmul(o_ps[:, po:po + 256], pa,
                                             rhs_l, start=False, stop=False)
                            nc.tensor.matmul(o_ps[:, po:po + 256], pb,
                                             rhs_h, start=False, stop=True)
                        # coeffs[l < m] == 0 structurally; evacuate only
                        # rows l >= lb (legal partition bases {0,64,96};
                        # output buffer is pre-zeroed)
                        lb = 96 if m8 >= 96 else (64 if m8 >= 64 else 0)
                        oo = (m4 - m8) * 256
                        if (m4 // MG) % 2 == 0:
                            nc.vector.tensor_copy(o_sb[lb:, oo:oo + 1024],
                                                  o_ps[lb:, :])
                        else:
                            nc.scalar.copy(o_sb[lb:, oo:oo + 1024],
                                           o_ps[lb:, :])
                    lb = 96 if m8 >= 96 else (64 if m8 >= 64 else 0)
                    eng = nc.sync if (m8 // OG) % 2 == 0 else nc.gpsimd
                    eng.dma_start(out=outp_d[lb:, m8:m8 + OG, :],
                                  in_=o_sb[lb:, :])

    nc.compile()
    return nc


_CACHE = {}


def _get_compiled():
    if "nc" not in _CACHE:
        _CACHE["nc"] = _build_bass()
    return _CACHE["nc"]


def kernel(data, Pw, E_re, E_im, pad_idx):
    from concourse import bass_utils

    data = np.asarray(data)
    Pw = np.asarray(Pw, dtype=np.float32)
    E_re = np.asarray(E_re, dtype=np.float32)
    E_im = np.asarray(E_im, dtype=np.float32)

    cores, nlon = _ring_assignment()
    offs = np.concatenate([[0], np.cumsum(nlon)[:-1]])
    # 'b e p v -> (b e v) p'
    x = np.ascontiguousarray(
        np.transpose(data, (0, 1, 3, 2)).reshape(BF, NPTS).astype(np.float32))
    PwT = np.ascontiguousarray(np.transpose(Pw, (1, 2, 0)))  # [m, n, l]

    in_maps = []
    for c in range(NCORES):
        xe, pw = _build_core_inputs(cores[c], nlon, offs, x, E_re, E_im, PwT)
        in_maps.append({"xe": xe, "pw": pw})

    nc = _get_compiled()
    res = bass_utils.run_bass_kernel_spmd(nc, in_maps, list(range(NCORES)))
    _CACHE["last_results"] = res

    total = np.zeros((LMAX, MMAX, 2 * BF), np.float64)
    for r in res.results:
        total += r["outp"].astype(np.float64)
    total = total.astype(np.float32).reshape(LMAX, MMAX, 2, BF)
    cc = total[:, :, 0, :] + 1j * total[:, :, 1, :]  # [l, m, bev]
    cc = cc.reshape(LMAX, MMAX, B, V)
    out = np.transpose(cc, (2, 0, 1, 3))[:, None]    # [b, 1, l, m, v]
    return out.astype(np.complex64)


# revision 59
# speedup vs baseline: 1.0148x; 1.0148x over previous
"""Octahedral SHT on 8 NeuronCores (Bass/Tile).

Strategy: shard the 192 latitude rings across 8 cores (24 rings each). Each
ring's ragged DFT (nlon in 20..400) is cut into K=128 chunks, zero-padded;
the ring assignment is engineered so every core gets exactly 51 chunks
-> one uniform SPMD program. The per-ring Legendre weights are replicated
per chunk, which folds the intra-ring chunk reduction into phase 2.
Each core computes a partial [l, m, bev] coefficient tensor over its own
rings; the host sums the 8 partials and assembles the complex output.

Precision: fp32 matmuls on the PE are 4x slower, so each fp32 operand is
split hi/lo into two fp16 tensors (x = hi + lo, |lo| <= 2^-11 |x|).
fp16*fp16 products are exact in the fp32 PSUM accumulator, so accumulating
MMs (hi*hi + hi*lo + lo*hi) reproduce the fp32 product to ~2^-22.

Phase 1 (per chunk c): G[c][m, (r,bev)] = E2[c].T @ x[c]  (PE, 6 MMs N=128,
  psum partition dim = m so the flatten yields m-major G' rows)
Flatten: G'[row c] <- [m, (hi|lo)] bounced through DRAM: SBUF->DRAM writes
  run at full HBM rate (the direct SBUF->SBUF gather is wall-limited by
  single-partition write bandwidth), then G' loads back in m-quarters whose
  SBUF writes spread across all 51 partitions, pipelined with phase-2 MMs.
  Row layout m*512 + h*256 + r*128 + bev.
Phase 2 (per m): out[l, (r,bev)] = 3 MMs (K=51):
  pw_hi @ G'hi (start) ; pw_hi @ G'lo ; pw_lo @ G'hi (stop)
"""
import numpy as np

NLAT, LMAX, MMAX = 192, 128, 128
B, V = 2, 64
BF = B * V            # 128 fused batch (b*64+v)
NCORES = 8
CHUNK = 128
NCH = 51              # chunks per core
RINGS_PER_CORE = 24
MAX_NLON = 400
NPTS = 40320
GB = [0, 4, 7, 10, 13, 17, 20, 23, 26, 30, 33, 36, 39, 43, 46, 49, NCH]
MG = 4                      # m's per psum tile
OG = 8                      # m's per out DMA group
PWG = 16                    # m's per pw/G' load group


def _octa_nlon():
    half = NLAT // 2
    north = np.array([4 * (i + 1) + 16 for i in range(half)], dtype=np.int64)
    return np.concatenate([north, north[::-1]])


def _ring_assignment():
    nlon = _octa_nlon()
    v = np.ceil(nlon / CHUNK).astype(int)
    cores = [[] for _ in range(NCORES)]
    for cls in (1, 2, 3, 4):
        ids = np.where(v == cls)[0]
        ids = ids[np.argsort(-nlon[ids], kind="stable")]
        fwd = True
        for start in range(0, len(ids), NCORES):
            blk = ids[start:start + NCORES]
            order = range(NCORES) if fwd else range(NCORES - 1, -1, -1)
            for c, rid in zip(order, blk):
                cores[c].append(int(rid))
            fwd = not fwd
    return cores, nlon


def _split16(a):
    hi = a.astype(np.float16)
    lo = (a - hi.astype(np.float32)).astype(np.float16)
    return hi, lo


def _build_core_inputs(core_rings, nlon, offs, x, E_re, E_im, PwT):
    """x: [BF, npts] f32.  Returns:
    xe  [128 j, 51 c, 768] f16  cols: [x_hi | x_lo | Ehi_re | Ehi_im | Elo_re | Elo_im]
    pw  [51, 128 m, 256] f16    cols: [pw_hi 0:128 | pw_lo 128:256]
    """
    xpad = np.zeros((NCH, CHUNK, BF), np.float32)
    E2 = np.zeros((NCH, CHUNK, 2 * MMAX), np.float32)
    Pw2 = np.zeros((MMAX, NCH, LMAX), np.float32)
    c = 0
    for r in core_rings:
        nl = int(nlon[r])
        o = int(offs[r])
        for j0 in range(0, nl, CHUNK):
            jlen = min(CHUNK, nl - j0)
            xpad[c, :jlen, :] = x[:, o + j0:o + j0 + jlen].T
            elen = min(CHUNK, MAX_NLON - j0)
            if elen > 0:
                E2[c, :elen, 0:MMAX] = E_re[r, j0:j0 + elen, :]
                E2[c, :elen, MMAX:] = E_im[r, j0:j0 + elen, :]
            Pw2[:, c, :] = PwT[:, r, :]
            c += 1
    assert c == NCH
    xh, xl = _split16(xpad)
    eh, el = _split16(E2)
    # eh/el cols: [re 0:128 | im 128:256]
    xe = np.concatenate([xh, xl, eh[:, :, 0:128], eh[:, :, 128:256],
                         el[:, :, 0:128], el[:, :, 128:256]], axis=2)
    xe = np.ascontiguousarray(xe.transpose(1, 0, 2))  # [128 j, 51 c, 768]

    ph, pl = _split16(Pw2)                           # [m, c, l] each
    pw = np.zeros((NCH, MMAX, 2 * LMAX), np.float16)
    pw[:, :, 0:128] = ph.transpose(1, 0, 2)          # pw_hi
    pw[:, :, 128:256] = pl.transpose(1, 0, 2)        # pw_lo
    return xe, pw


def _build_bass():
    import concourse.bass as bass
    import concourse.mybir as mybir
    from concourse import bacc, tile

    dt = mybir.dt
    nc = bacc.Bacc()

    xe_d = nc.dram_tensor("xe", [CHUNK, NCH, 768], dt.float16,
                          kind="ExternalInput")
    pw_d = nc.dram_tensor("pw", [NCH, MMAX, 2 * LMAX], dt.float16,
                          kind="ExternalInput")
    outp_d = nc.dram_tensor("outp", [LMAX, MMAX, 2 * BF], dt.float32,
                            kind="ExternalOutput")
    gdram = nc.dram_tensor("gdram", [NCH, MMAX * 512], dt.float16)

    with tile.TileContext(nc) as tc:
        with (
            tc.tile_pool(name="xs", bufs=4) as xs_pool,
            tc.tile_pool(name="gt", bufs=4) as gt_pool,
            tc.tile_pool(name="gs", bufs=4) as gs_pool,
            tc.tile_pool(name="pws", bufs=4) as pw_pool,
            tc.tile_pool(name="os", bufs=2) as os_pool,
            tc.tile_pool(name="ps1", bufs=2, space="PSUM") as ps1,
            tc.tile_pool(name="ps2", bufs=3, space="PSUM") as ps2,
        ):
            # ---- phase 1: 51 chunks x 6 accumulating MMs ----
            NG = len(GB) - 1
            xg = {}
            for g in range(NG):
                n = GB[g + 1] - GB[g]
                t = xs_pool.tile([CHUNK, n * 768], dt.float16, tag="xg")
                # split by partition halves: 2 parallel DMA engines per group
                nc.sync.dma_start(out=t[0:64, :],
                                  in_=xe_d[0:64, GB[g]:GB[g + 1], :])
                nc.scalar.dma_start(out=t[64:128, :],
                                    in_=xe_d[64:128, GB[g]:GB[g + 1], :])
                xg[g] = t

            for c in range(NCH):
                g = next(i for i in range(NG) if GB[i] <= c < GB[i + 1])
                off = (c - GB[g]) * 768
                xe = xg[g]
                xh = xe[:, off + 0:off + 128]
                xl = xe[:, off + 128:off + 256]
                ehr = xe[:, off + 256:off + 384]
                ehi = xe[:, off + 384:off + 512]
                elr = xe[:, off + 512:off + 640]
                eli = xe[:, off + 640:off + 768]
                # psum [m, (re_bev | im_bev)]; 6 MMs, lhsT = E slices
                g_ps = ps1.tile([MMAX, 2 * BF], dt.float32, tag="g")
                re = g_ps[:, 0:128]
                im = g_ps[:, 128:256]
                nc.tensor.matmul(re, ehr, xh, start=True, stop=False)
                nc.tensor.matmul(re, ehr, xl, start=False, stop=False)
                nc.tensor.matmul(re, elr, xh, start=False, stop=True)
                nc.tensor.matmul(im, ehi, xh, start=True, stop=False)
                nc.tensor.matmul(im, ehi, xl, start=False, stop=False)
                nc.tensor.matmul(im, eli, xh, start=False, stop=True)
                # evacuate PSUM, splitting fp32 -> fp16 hi (ACT) + lo (DVE)
                g_hl = gt_pool.tile([MMAX, 512], dt.float16, tag="ghl")
                nc.scalar.copy(g_hl[:, 0:256], g_ps[:])
                nc.vector.tensor_sub(g_hl[:, 256:512], g_ps[:], g_hl[:, 0:256])
                # flatten to DRAM (full-rate HBM write, 1KB runs)
                nc.gpsimd.dma_start(out=gdram[c], in_=g_hl[:])

            # ---- phase 2: 128 m x 3 accumulating MMs ----
            for mg in range(0, MMAX, PWG):
                pwt = pw_pool.tile([NCH, PWG * 256], dt.float16, tag="pw")
                nc.sync.dma_start(out=pwt[:], in_=pw_d[:, mg:mg + PWG, :])
                gsb = gs_pool.tile([NCH, PWG * 512], dt.float16, tag="gq")
                # parallel loads (8KB runs each) split across queues; the
                # first group is on the critical path -> split 4 ways
                nsplit = 4 if mg == 0 else 2
                step = PWG * 512 // nsplit
                engs = (nc.sync, nc.gpsimd, nc.scalar, nc.sync)
                for si in range(nsplit):
                    engs[si].dma_start(
                        out=gsb[:, si * step:(si + 1) * step],
                        in_=gdram[:, mg * 512 + si * step:
                                  mg * 512 + (si + 1) * step])
                for m8 in range(mg, mg + PWG, OG):
                    o_sb = os_pool.tile([LMAX, OG * 256], dt.float32, tag="ot")
                    for m4 in range(m8, m8 + OG, MG):
                        o_ps = ps2.tile([LMAX, MG * 256], dt.float32, tag="o")
                        for m in range(m4, m4 + MG):
                            mo = (m - mg) * 256
                            po = (m - m4) * 256
                            ml = (m - mg) * 512
                            pa = pwt[:, mo + 0:mo + 128]
                            pb = pwt[:, mo + 128:mo + 256]
                            rhs_h = gsb[:, ml:ml + 256]
                            rhs_l = gsb[:, ml + 256:ml + 512]
                            nc.tensor.matmul(o_ps[:, po:po + 256], pa,
                                             rhs_h, start=True, stop=False)
                            nc.tensor.matmul(o_ps[:, po:po + 256], pa,
                                             rhs_l, start=False, stop=False)
                            nc.tensor.matmul(o_ps[:, po:po + 256], pb,
                                             rhs_h, start=False, stop=True)
                        # coeffs[l < m] == 0 structurally; evacuate only
                        # rows l >= lb (legal partition bases {0,64,96};
                        # output buffer is pre-zeroed)
                        lb = 96 if m8 >= 96 else (64 if m8 >= 64 else 0)
                        oo = (m4 - m8) * 256
                        if (m4 // MG) % 2 == 0:
                            nc.vector.tensor_copy(o_sb[lb:, oo:oo + 1024],
                                                  o_ps[lb:, :])
                        else:
                            nc.scalar.copy(o_sb[lb:, oo:oo + 1024],
                                           o_ps[lb:, :])
                    lb = 96 if m8 >= 96 else (64 if m8 >= 64 else 0)
                    eng = nc.sync if (m8 // OG) % 2 == 0 else nc.gpsimd
                    eng.dma_start(out=outp_d[lb:, m8:m8 + OG, :],
                                  in_=o_sb[lb:, :])

    nc.compile()
    return nc


_CACHE = {}


def _get_compiled():
    if "nc" not in _CACHE:
        _CACHE["nc"] = _build_bass()
    return _CACHE["nc"]


def kernel(data, Pw, E_re, E_im, pad_idx):
    from concourse import bass_utils

    data = np.asarray(data)
    Pw = np.asarray(Pw, dtype=np.float32)
    E_re = np.asarray(E_re, dtype=np.float32)
    E_im = np.asarray(E_im, dtype=np.float32)

    cores, nlon = _ring_assignment()
    offs = np.concatenate([[0], np.cumsum(nlon)[:-1]])
    # 'b e p v -> (b e v) p'
    x = np.ascontiguousarray(
        np.transpose(data, (0, 1, 3, 2)).reshape(BF, NPTS).astype(np.float32))
    PwT = np.ascontiguousarray(np.transpose(Pw, (1, 2, 0)))  # [m, n, l]

    in_maps = []
    for c in range(NCORES):
        xe, pw = _build_core_inputs(cores[c], nlon, offs, x, E_re, E_im, PwT)
        in_maps.append({"xe": xe, "pw": pw})

    nc = _get_compiled()
    res = bass_utils.run_bass_kernel_spmd(nc, in_maps, list(range(NCORES)))
    _CACHE["last_results"] = res

    total = np.zeros((LMAX, MMAX, 2 * BF), np.float64)
    for r in res.results:
        total += r["outp"].astype(np.float64)
    total = total.astype(np.float32).reshape(LMAX, MMAX, 2, BF)
    cc = total[:, :, 0, :] + 1j * total[:, :, 1, :]  # [l, m, bev]
    cc = cc.reshape(LMAX, MMAX, B, V)
    out = np.transpose(cc, (2, 0, 1, 3))[:, None]    # [b, 1, l, m, v]
    return out.astype(np.complex64)


# revision 60
# speedup vs baseline: 1.0521x; 1.0367x over previous
"""Octahedral SHT on 8 NeuronCores (Bass/Tile).

Strategy: shard the 192 latitude rings across 8 cores (24 rings each). Each
ring's ragged DFT (nlon in 20..400) is cut into K=128 chunks, zero-padded;
the ring assignment is engineered so every core gets exactly 51 chunks
-> one uniform SPMD program. The per-ring Legendre weights are replicated
per chunk, which folds the intra-ring chunk reduction into phase 2.
Each core computes a partial [l, m, bev] coefficient tensor over its own
rings; the host sums the 8 partials and assembles the complex output.

Precision: fp32 matmuls on the PE are 4x slower, so each fp32 operand is
split hi/lo into two fp16 tensors (x = hi + lo, |lo| <= 2^-11 |x|).
fp16*fp16 products are exact in the fp32 PSUM accumulator, so accumulating
MMs (hi*hi + hi*lo + lo*hi) reproduce the fp32 product to ~2^-22.

Phase 1 (per chunk c): G[c][m, (r,bev)] = E2[c].T @ x[c]  (PE, 6 MMs N=128,
  psum partition dim = m so the flatten yields m-major G' rows)
Flatten: G'[row c] <- [m, (hi|lo)] bounced through DRAM: SBUF->DRAM writes
  run at full HBM rate (the direct SBUF->SBUF gather is wall-limited by
  single-partition write bandwidth), then G' loads back in m-quarters whose
  SBUF writes spread across all 51 partitions, pipelined with phase-2 MMs.
  Row layout m*512 + h*256 + r*128 + bev.
Phase 2 (per m): out[l, (r,bev)] = 3 MMs (K=51):
  pw_hi @ G'hi (start) ; pw_hi @ G'lo ; pw_lo @ G'hi (stop)
"""
import numpy as np

NLAT, LMAX, MMAX = 192, 128, 128
B, V = 2, 64
BF = B * V            # 128 fused batch (b*64+v)
NCORES = 8
CHUNK = 128
NCH = 51              # chunks per core
RINGS_PER_CORE = 24
MAX_NLON = 400
NPTS = 40320
GB = [0, 4, 7, 10, 13, 17, 20, 23, 26, 30, 33, 36, 39, 43, 46, 49, NCH]
MG = 4                      # m's per psum tile
OG = 8                      # m's per out DMA group
PWG = 16                    # m's per pw/G' load group


def _octa_nlon():
    half = NLAT // 2
    north = np.array([4 * (i + 1) + 16 for i in range(half)], dtype=np.int64)
    return np.concatenate([north, north[::-1]])


def _ring_assignment():
    nlon = _octa_nlon()
    v = np.ceil(nlon / CHUNK).astype(int)
    cores = [[] for _ in range(NCORES)]
    for cls in (1, 2, 3, 4):
        ids = np.where(v == cls)[0]
        ids = ids[np.argsort(-nlon[ids], kind="stable")]
        fwd = True
        for start in range(0, len(ids), NCORES):
            blk = ids[start:start + NCORES]
            order = range(NCORES) if fwd else range(NCORES - 1, -1, -1)
            for c, rid in zip(order, blk):
                cores[c].append(int(rid))
            fwd = not fwd
    return cores, nlon


def _split16(a):
    hi = a.astype(np.float16)
    lo = (a - hi.astype(np.float32)).astype(np.float16)
    return hi, lo


def _build_core_inputs(core_rings, nlon, offs, x, E_re, E_im, PwT):
    """x: [BF, npts] f32.  Returns:
    xe  [128 j, 51 c, 768] f16  cols: [x_hi | x_lo | Ehi_re | Ehi_im | Elo_re | Elo_im]
    pw  [51, 128 m, 256] f16    cols: [pw_hi 0:128 | pw_lo 128:256]
    """
    xpad = np.zeros((NCH, CHUNK, BF), np.float32)
    E2 = np.zeros((NCH, CHUNK, 2 * MMAX), np.float32)
    Pw2 = np.zeros((MMAX, NCH, LMAX), np.float32)
    c = 0
    for r in core_rings:
        nl = int(nlon[r])
        o = int(offs[r])
        for j0 in range(0, nl, CHUNK):
            jlen = min(CHUNK, nl - j0)
            xpad[c, :jlen, :] = x[:, o + j0:o + j0 + jlen].T
            elen = min(CHUNK, MAX_NLON - j0)
            if elen > 0:
                E2[c, :elen, 0:MMAX] = E_re[r, j0:j0 + elen, :]
                E2[c, :elen, MMAX:] = E_im[r, j0:j0 + elen, :]
            Pw2[:, c, :] = PwT[:, r, :]
            c += 1
    assert c == NCH
    xh, xl = _split16(xpad)
    eh, el = _split16(E2)
    # eh/el cols: [re 0:128 | im 128:256]
    xe = np.concatenate([xh, xl, eh[:, :, 0:128], eh[:, :, 128:256],
                         el[:, :, 0:128], el[:, :, 128:256]], axis=2)
    xe = np.ascontiguousarray(xe.transpose(1, 0, 2))  # [128 j, 51 c, 768]

    ph, pl = _split16(Pw2)                           # [m, c, l] each
    pw = np.zeros((NCH, MMAX, 2 * LMAX), np.float16)
    pw[:, :, 0:128] = ph.transpose(1, 0, 2)          # pw_hi
    pw[:, :, 128:256] = pl.transpose(1, 0, 2)        # pw_lo
    return xe, pw


def _build_bass():
    import concourse.bass as bass
    import concourse.mybir as mybir
    from concourse import bacc, tile

    dt = mybir.dt
    nc = bacc.Bacc()

    xe_d = nc.dram_tensor("xe", [CHUNK, NCH, 768], dt.float16,
                          kind="ExternalInput")
    pw_d = nc.dram_tensor("pw", [NCH, MMAX, 2 * LMAX], dt.float16,
                          kind="ExternalInput")
    outp_d = nc.dram_tensor("outp", [LMAX, MMAX, 2 * BF], dt.float32,
                            kind="ExternalOutput")
    gdram = nc.dram_tensor("gdram", [NCH, MMAX * 512], dt.float16)

    with tile.TileContext(nc) as tc:
        with (
            tc.tile_pool(name="xs", bufs=4) as xs_pool,
            tc.tile_pool(name="gt", bufs=4) as gt_pool,
            tc.tile_pool(name="gs", bufs=4) as gs_pool,
            tc.tile_pool(name="pws", bufs=4) as pw_pool,
            tc.tile_pool(name="os", bufs=2) as os_pool,
            tc.tile_pool(name="ps1", bufs=2, space="PSUM") as ps1,
            tc.tile_pool(name="ps2", bufs=3, space="PSUM") as ps2,
        ):
            # ---- phase 1: 51 chunks x 6 accumulating MMs ----
            NG = len(GB) - 1
            xg = {}
            for g in range(NG):
                n = GB[g + 1] - GB[g]
                t = xs_pool.tile([CHUNK, n * 768], dt.float16, tag="xg")
                # split by partition halves: 2 parallel DMA engines per group
                nc.sync.dma_start(out=t[0:64, :],
                                  in_=xe_d[0:64, GB[g]:GB[g + 1], :])
                nc.scalar.dma_start(out=t[64:128, :],
                                    in_=xe_d[64:128, GB[g]:GB[g + 1], :])
                xg[g] = t

            for c in range(NCH):
                g = next(i for i in range(NG) if GB[i] <= c < GB[i + 1])
                off = (c - GB[g]) * 768
                xe = xg[g]
                xh = xe[:, off + 0:off + 128]
                xl = xe[:, off + 128:off + 256]
                ehr = xe[:, off + 256:off + 384]
                ehi = xe[:, off + 384:off + 512]
                elr = xe[:, off + 512:off + 640]
                eli = xe[:, off + 640:off + 768]
                # psum [m, (re_bev | im_bev)]; 6 MMs, lhsT = E slices
                g_ps = ps1.tile([MMAX, 2 * BF], dt.float32, tag="g")
                re = g_ps[:, 0:128]
                im = g_ps[:, 128:256]
                nc.tensor.matmul(re, ehr, xh, start=True, stop=False)
                nc.tensor.matmul(re, ehr, xl, start=False, stop=False)
                nc.tensor.matmul(re, elr, xh, start=False, stop=True)
                nc.tensor.matmul(im, ehi, xh, start=True, stop=False)
                nc.tensor.matmul(im, ehi, xl, start=False, stop=False)
                nc.tensor.matmul(im, eli, xh, start=False, stop=True)
                # evacuate PSUM, splitting fp32 -> fp16 hi (ACT) + lo (DVE)
                g_hl = gt_pool.tile([MMAX, 512], dt.float16, tag="ghl")
                nc.scalar.copy(g_hl[:, 0:256], g_ps[:])
                nc.vector.tensor_sub(g_hl[:, 256:512], g_ps[:], g_hl[:, 0:256])
                # flatten to DRAM (full-rate HBM write, 1KB runs)
                nc.gpsimd.dma_start(out=gdram[c], in_=g_hl[:])

            # ---- phase 2: 128 m x 3 accumulating MMs ----
            for mg in range(0, MMAX, PWG):
                pwt = pw_pool.tile([NCH, PWG * 256], dt.float16, tag="pw")
                nc.sync.dma_start(out=pwt[:], in_=pw_d[:, mg:mg + PWG, :])
                gsb = gs_pool.tile([NCH, PWG * 512], dt.float16, tag="gq")
                # two parallel loads (8KB runs each), split across queues
                half = PWG * 256
                nc.sync.dma_start(
                    out=gsb[:, 0:half],
                    in_=gdram[:, mg * 512:mg * 512 + half])
                nc.gpsimd.dma_start(
                    out=gsb[:, half:2 * half],
                    in_=gdram[:, mg * 512 + half:(mg + PWG) * 512])
                for m8 in range(mg, mg + PWG, OG):
                    o_sb = os_pool.tile([LMAX, OG * 256], dt.float32, tag="ot")
                    for m4 in range(m8, m8 + OG, MG):
                        o_ps = ps2.tile([LMAX, MG * 256], dt.float32, tag="o")
                        for m in range(m4, m4 + MG):
                            mo = (m - mg) * 256
                            po = (m - m4) * 256
                            ml = (m - mg) * 512
                            pa = pwt[:, mo + 0:mo + 128]
                            pb = pwt[:, mo + 128:mo + 256]
                            rhs_h = gsb[:, ml:ml + 256]
                            rhs_l = gsb[:, ml + 256:ml + 512]
                            nc.tensor.matmul(o_ps[:, po:po + 256], pa,
                                             rhs_h, start=True, stop=False)
                            nc.tensor.matmul(o_ps[:, po:po + 256], pa,
                                             rhs_l, start=False, stop=False)
                            nc.tensor.matmul(o_ps[:, po:po + 256], pb,
                                             rhs_h, start=False, stop=True)
                        # coeffs[l < m] == 0 structurally; evacuate only
                        # rows l >= lb (legal partition bases {0,64,96};
                        # output buffer is pre-zeroed)
                        lb = 96 if m8 >= 96 else (64 if m8 >= 64 else 0)
                        oo = (m4 - m8) * 256
                        if (m4 // MG) % 2 == 0:
                            nc.vector.tensor_copy(o_sb[lb:, oo:oo + 1024],
                                                  o_ps[lb:, :])
                        else:
                            nc.scalar.copy(o_sb[lb:, oo:oo + 1024],
                                           o_ps[lb:, :])
                    lb = 96 if m8 >= 96 else (64 if m8 >= 64 else 0)
                    eng = nc.sync if (m8 // OG) % 2 == 0 else nc.gpsimd
                    eng.dma_start(out=outp_d[lb:, m8:m8 + OG, :],
                                  in_=o_sb[lb:, :])

    nc.compile()
    return nc


_CACHE = {}


def _get_compiled():
    if "nc" not in _CACHE:
        _CACHE["nc"] = _build_bass()
    return _CACHE["nc"]


def kernel(data, Pw, E_re, E_im, pad_idx):
    from concourse import bass_utils

    data = np.asarray(data)
    Pw = np.asarray(Pw, dtype=np.float32)
    E_re = np.asarray(E_re, dtype=np.float32)
    E_im = np.asarray(E_im, dtype=np.float32)

    cores, nlon = _ring_assignment()
    offs = np.concatenate([[0], np.cumsum(nlon)[:-1]])
    # 'b e p v -> (b e v) p'
    x = np.ascontiguousarray(
        np.transpose(data, (0, 1, 3, 2)).reshape(BF, NPTS).astype(np.float32))
    PwT = np.ascontiguousarray(np.transpose(Pw, (1, 2, 0)))  # [m, n, l]

    in_maps = []
    for c in range(NCORES):
        xe, pw = _build_core_inputs(cores[c], nlon, offs, x, E_re, E_im, PwT)
        in_maps.append({"xe": xe, "pw": pw})

    nc = _get_compiled()
    res = bass_utils.run_bass_kernel_spmd(nc, in_maps, list(range(NCORES)))
    _CACHE["last_results"] = res

    total = np.zeros((LMAX, MMAX, 2 * BF), np.float64)
    for r in res.results:
        total += r["outp"].astype(np.float64)
    total = total.astype(np.float32).reshape(LMAX, MMAX, 2, BF)
    cc = total[:, :, 0, :] + 1j * total[:, :, 1, :]  # [l, m, bev]
    cc = cc.reshape(LMAX, MMAX, B, V)
    out = np.transpose(cc, (2, 0, 1, 3))[:, None]    # [b, 1, l, m, v]
    return out.astype(np.complex64)
